# revision 21
# baseline (speedup 1.0000x reference)
"""Trainium2 Bass kernel for the Dynamic MultiTeacher4 distillation loss.

Strategy (pure data parallel over the batch):
  - B=8192 rows are sharded 1024/core across 8 NeuronCores; 8 row-blocks
    of 128 partitions per core.
  - Per row the device emits 17 reduction stats:
      M1..M4, Mm : row maxes of bf16(exp(o_t/20)) and bf16(exp(mimic4/80)).
                   exp is monotone, so the host recovers EXACT margin /
                   max_preds semantics from these with a small-tolerance
                   candidate search (a handful of rows recomputed exactly
                   on host).
      A1..A4, Am : sum_c exp(t/20)  (A4, Am ride free on ACT accum_out;
                   A1..A3 are DVE bf16 tensor_scalar sum-reduces at 4x)
      B1..B4, Bm : sum_c exp(t/20)*s  (bf16 DVE mult at 2x + bf16
                   tensor_scalar sum-reduce at 4x; the mimic dot usually
                   runs on GPSIMD, one entry lagged, to unload DVE)
      S1, S2     : sum_c exp(s), sum_c exp(s/20)
  - ACT cost is elems*0.83ns + ~370ns/instruction, so the teacher exps,
    exp(s/20) and exp(s) are fused into ONE wide activation over a
    [128, 5w] concatenated tile [o1|o2|o3|s|sk] at a single scale 1/20:
    GPSIMD precomputes sk = 20*s so exp(sk/20) == exp(s). e4 stays a
    separate instruction to keep its free accum (A4), as does
    em = exp(pm/80) (accum Am).
  - mimic4 = ((o1+o2)+o3)+o4 runs on the TensorEngine as 4 identity
    matmuls accumulating into PSUM, with operands bitcast to float32r:
    1 cycle/row instead of f32's 4 (the interpreter computes f32r
    exactly like f32, so pm stays bit-exact).
  - Block 0 uses per-tensor activations (baseline structure) so compute
    starts ~2.6us in instead of waiting for 4 tensors + a GPSIMD pass;
    the final block is processed in column halves to shorten the
    DMA-end -> stats critical chain.

Per-core HBM traffic is the 20.5MB of inputs; DMA is the roofline at
~57us and every compute engine is scheduled under it.
"""

import os
import time

import numpy as np

import concourse.bass as bass
import concourse.bacc as bacc
import concourse.tile as tile
from concourse import mybir
from concourse.bass_utils import run_bass_kernel_spmd
from concourse.masks import make_identity

B, C = 8192, 1000
NCORES = 8
ROWS = B // NCORES  # 1024 rows per core
P = 128
NBLK = ROWS // P  # 8 row-blocks per core

ALPHA = 0.8
T_KD = 20.0
T_THR = 2.0

# ramp/tail shaping (tunable)
SEP_N = int(os.environ.get("KERNEL_SEP_N", "1"))  # leading blocks in separate-exp mode
TAIL_HALVES = int(os.environ.get("KERNEL_TAIL_HALVES", "1"))  # trailing blocks split into C-halves

_NC = None
LAST_RESULTS = None  # BassKernelResults of the most recent run (for profiling)

# tensor order inside the concatenated tile: [o1|o2|o3|s|sk|o4]
SL_O1, SL_O2, SL_O3, SL_S, SL_SK, SL_O4 = range(6)
# st_dve columns
D_M = 0  # 0..4: M1..M4, Mm
D_A = 5  # 5..7: A1..A3
D_S1 = 8
D_S2 = 9
D_B = 10  # 10..13: B1..B4


def _entries():
    """(row_block, c0, width, slot, mode) for every virtual block."""
    ents = []
    slot = 0
    for i in range(NBLK):
        mode = "sep" if i < SEP_N else "wide"
        if i >= NBLK - TAIL_HALVES and mode == "wide":
            ents.append((i, 0, C // 2, slot, mode))
            ents.append((i, C // 2, C // 2, slot + 1, mode))
            slot += 2
        else:
            ents.append((i, 0, C, slot, mode))
            slot += 1
    return ents


ENTRIES = _entries()
NSLOT = len(ENTRIES)


def _build():
    f32 = mybir.dt.float32
    f32r = mybir.dt.float32r
    bf16 = mybir.dt.bfloat16
    Alu = mybir.AluOpType
    Act = mybir.ActivationFunctionType

    nc = bacc.Bacc(
        "TRN2", target_bir_lowering=False, debug=False, num_devices=NCORES
    )

    o1 = nc.dram_tensor("o1", [ROWS, C], f32, kind="ExternalInput").ap()
    o2 = nc.dram_tensor("o2", [ROWS, C], f32, kind="ExternalInput").ap()
    o3 = nc.dram_tensor("o3", [ROWS, C], f32, kind="ExternalInput").ap()
    o4 = nc.dram_tensor("o4", [ROWS, C], f32, kind="ExternalInput").ap()
    s_ = nc.dram_tensor("s", [ROWS, C], f32, kind="ExternalInput").ap()
    st_out = nc.dram_tensor("st_out", [NSLOT, P, 17], f32, kind="ExternalOutput").ap()

    o1r = o1.rearrange("(n p) c -> n p c", p=P)
    o2r = o2.rearrange("(n p) c -> n p c", p=P)
    o3r = o3.rearrange("(n p) c -> n p c", p=P)
    o4r = o4.rearrange("(n p) c -> n p c", p=P)
    sr = s_.rearrange("(n p) c -> n p c", p=P)
    dram = {SL_O1: o1r, SL_O2: o2r, SL_O3: o3r, SL_O4: o4r, SL_S: sr}

    with tile.TileContext(nc) as tc:
        with (
            tc.tile_pool(name="const", bufs=1) as const,
            tc.tile_pool(name="io", bufs=4) as io,
            tc.tile_pool(name="wk", bufs=3) as wk,
            tc.tile_pool(name="st", bufs=NSLOT + 1) as st,
            tc.tile_pool(name="ps", bufs=3, space="PSUM") as ps,
        ):
            ident = const.tile([P, P], f32, tag="ident")
            make_identity(nc, ident)
            # the BIR verifier requires fp32r matmul operands to come from
            # fp32r-typed producers
            identr = const.tile([P, P], f32r, tag="identr")
            nc.gpsimd.tensor_copy(out=identr, in_=ident)
            # warm the PE out of its cold p-state during the DMA-paced ramp
            warm = ps.tile(
                [P, 2, 500], f32, tag="pm2", padded_shape=[P, 2, 512], bufs=3
            )
            for _ in range(6):
                nc.tensor.matmul(
                    warm[:, 0, 0:P], ident, ident, start=True, stop=True
                )

            stats_tiles = []
            pending_pool_dots = []  # (in0, in1, accum_dst, width, tag)

            for i, c0, w, slot, mode in ENTRIES:
                nh = max(1, w // 500)
                H = w // nh
                # inA holds the 4 teachers (every writer fp32r-typed for the
                # BIR verifier's fp32r-matmul producer check; the DMA bitcasts
                # are pure bit copies so the f32 values are untouched).
                # inB = [s|sk] has no matmul consumers, so GPSIMD may write sk.
                inA = io.tile([P, 4 * w], f32, tag=f"inA{w}")
                inB = io.tile([P, 2 * w], f32, tag=f"inB{w}")
                ecA = wk.tile([P, 4 * w], bf16, tag=f"ecA{w}")
                ecB = wk.tile([P, 2 * w], bf16, tag=f"ecB{w}")
                stt = st.tile([P, 17], f32, tag="stt")
                sd = stt[:, 0:14]
                sa = stt[:, 14:16]
                sg = stt[:, 16:17]

                def slA(t, tile_=None):
                    tl = inA if tile_ is None else tile_
                    return tl[:, t * w : (t + 1) * w]

                s_in = inB[:, 0:w]
                sk_in = inB[:, w : 2 * w]
                es20 = ecB[:, 0:w]
                es = ecB[:, w : 2 * w]

                # ---- loads, issue spread over the SP/ACT/DVE queues so no
                # single sequencer's ~650ns/issue HWDGE occupancy paces the
                # stream ----
                if mode == "sep":
                    t_order = (0, None, 1, 2, 3)
                else:
                    t_order = (None, 0, 1, 2, 3)
                qs = (nc.sync, nc.scalar, nc.sync, nc.scalar, nc.sync)
                for q, t in zip(qs, t_order):
                    if t is None:
                        q.dma_start(out=s_in, in_=dram[SL_S][i][:, c0 : c0 + w])
                    else:
                        q.dma_start(
                            out=slA(t).bitcast(f32r),
                            in_=dram[t][i][:, c0 : c0 + w].bitcast(f32r),
                        )

                # ---- GPSIMD: sk = 20*s (wide mode), sb = bf16(s), and the
                # previous entry's lagged dot PRODUCTS (Pool has no accum
                # support in codegen; the paired reduces run on DVE below) ----
                sb = wk.tile([P, w], bf16, tag=f"sb{w}")
                if mode == "wide":
                    nc.gpsimd.tensor_scalar(
                        out=sk_in, in0=s_in, scalar1=20.0, scalar2=None,
                        op0=Alu.mult, op1=Alu.bypass,
                    )
                nc.gpsimd.tensor_copy(out=sb, in_=s_in)
                lagged_reduces = []  # (product_tile, accum_slice)
                for em_p, sb_p, dst_p, w_p, tagn in pending_pool_dots:
                    pp = wk.tile([P, w_p], bf16, tag=f"pp{tagn}{w_p}", bufs=2)
                    nc.gpsimd.tensor_mul(out=pp, in0=em_p, in1=sb_p)
                    lagged_reduces.append((pp, dst_p))
                pending_pool_dots = []

                # ---- PE: pm = ((o1+o2)+o3)+o4 in f32r (exact in interp) ----
                pm = ps.tile(
                    [P, nh, H], f32, tag=f"pm{nh}",
                    padded_shape=[P, nh, 512], bufs=(3 if nh == 2 else 2),
                )
                for j in range(nh):
                    for k in range(4):
                        nc.tensor.matmul(
                            pm[:, j, :],
                            identr,
                            inA[:, k * w + j * H : k * w + (j + 1) * H].bitcast(f32r),
                            start=(k == 0),
                            stop=(k == 3),
                        )

                # ---- ACT ----
                em = wk.tile([P, w], bf16, tag=f"em{w}")
                if mode == "sep":
                    # per-tensor exps, accum sums ride free
                    for k in range(3):
                        nc.scalar.activation(
                            out=slA(k, ecA), in_=slA(k), func=Act.Exp,
                            scale=1.0 / T_KD,
                            accum_out=sd[:, D_A + k : D_A + k + 1],
                        )
                    nc.scalar.activation(
                        out=slA(3, ecA), in_=slA(3), func=Act.Exp,
                        scale=1.0 / T_KD, accum_out=sa[:, 0:1],
                    )
                    nc.scalar.activation(
                        out=em.rearrange("p (j c) -> p j c", j=nh), in_=pm,
                        func=Act.Exp, scale=1.0 / (4.0 * T_KD),
                        accum_out=sa[:, 1:2],
                    )
                    nc.scalar.activation(
                        out=es20, in_=s_in, func=Act.Exp,
                        scale=1.0 / T_KD, accum_out=sd[:, D_S2 : D_S2 + 1],
                    )
                    nc.scalar.activation(
                        out=es, in_=s_in, func=Act.Exp,
                        scale=1.0, accum_out=sd[:, D_S1 : D_S1 + 1],
                    )
                else:
                    # wide exp over [o1|o2|o3] and [s|sk], both at scale 1/20
                    nc.scalar.activation(
                        out=ecA[:, 0 : 3 * w], in_=inA[:, 0 : 3 * w],
                        func=Act.Exp, scale=1.0 / T_KD,
                    )
                    nc.scalar.activation(
                        out=ecB, in_=inB, func=Act.Exp, scale=1.0 / T_KD,
                    )
                    nc.scalar.activation(
                        out=slA(3, ecA), in_=slA(3), func=Act.Exp,
                        scale=1.0 / T_KD, accum_out=sa[:, 0:1],
                    )
                    nc.scalar.activation(
                        out=em.rearrange("p (j c) -> p j c", j=nh), in_=pm,
                        func=Act.Exp, scale=1.0 / (4.0 * T_KD),
                        accum_out=sa[:, 1:2],
                    )

                # ---- DVE ----
                scrap = wk.tile([P, w], bf16, tag=f"sc{w}", bufs=2)
                prod = wk.tile([P, w], bf16, tag=f"pr{w}", bufs=2)

                def dve_max(src, col):
                    nc.vector.tensor_scalar(
                        out=scrap, in0=src, scalar1=1.0, scalar2=None,
                        op0=Alu.mult, op1=Alu.max,
                        accum_out=sd[:, col : col + 1],
                    )

                def dve_sum(src, dst):
                    nc.vector.tensor_scalar(
                        out=scrap, in0=src, scalar1=1.0, scalar2=None,
                        op0=Alu.mult, op1=Alu.add, accum_out=dst,
                    )

                def dve_dot(src, col):
                    nc.vector.tensor_mul(out=prod, in0=src, in1=sb)
                    nc.vector.tensor_scalar(
                        out=scrap, in0=prod, scalar1=1.0, scalar2=None,
                        op0=Alu.mult, op1=Alu.add,
                        accum_out=sd[:, col : col + 1],
                    )

                # previous entry's lagged reduces first (their Pool products
                # are ready early in this block; widths may differ from w)
                for pp, dst in lagged_reduces:
                    w_p = pp.shape[1]
                    scl = wk.tile([P, w_p], bf16, tag=f"scl{w_p}", bufs=2)
                    nc.vector.tensor_scalar(
                        out=scl, in0=pp, scalar1=1.0, scalar2=None,
                        op0=Alu.mult, op1=Alu.add, accum_out=dst,
                    )

                for k in range(3):
                    dve_max(slA(k, ecA), D_M + k)
                    if mode == "wide":
                        dve_sum(slA(k, ecA), sd[:, D_A + k : D_A + k + 1])
                    dve_dot(slA(k, ecA), D_B + k)
                if mode == "wide":
                    dve_sum(es, sd[:, D_S1 : D_S1 + 1])
                    dve_sum(es20, sd[:, D_S2 : D_S2 + 1])
                dve_max(slA(3, ecA), D_M + 3)
                last = slot == NSLOT - 1
                if not last and slot % 2 == 0:
                    # lag the e4 dot product onto Pool on even slots
                    pending_pool_dots.append(
                        (slA(3, ecA), sb, sd[:, D_B + 3 : D_B + 4], w, "e4")
                    )
                else:
                    dve_dot(slA(3, ecA), D_B + 3)
                dve_max(em, D_M + 4)
                if last:
                    # tail: run the mimic dot on DVE instead of lagging it
                    nc.vector.tensor_mul(out=prod, in0=em, in1=sb)
                    nc.vector.tensor_scalar(
                        out=scrap, in0=prod, scalar1=1.0, scalar2=None,
                        op0=Alu.mult, op1=Alu.add, accum_out=sg[:, 0:1],
                    )
                else:
                    pending_pool_dots.append((em, sb, sg[:, 0:1], w, "m"))

                stats_tiles.append((slot, stt))

            # stats stores after the loop: never block a later block's loads
            # behind a store on the in-order sync queue
            for slot, stt in stats_tiles:
                nc.sync.dma_start(out=st_out[slot], in_=stt)

    nc.compile()
    return nc


def _get_nc():
    global _NC
    if _NC is None:
        _NC = _build()
    return _NC


def _merge_slots(arr, op):
    """[NSLOT, P, K] per-slot stats -> [NBLK*P, K] per-row stats."""
    out = []
    for i in range(NBLK):
        slots = [s for (ib, _c0, _w, s, _m) in ENTRIES if ib == i]
        m = arr[slots[0]]
        for s in slots[1:]:
            m = op(m, arr[s])
        out.append(m)
    return np.concatenate(out, 0)


def gather_stats(res):
    """Merge per-slot device stats into per-row [B, *] arrays.

    Returns (sd, sa, sg): sd[:,0:5]=M maxes, sd[:,5:8]=A1..3, sd[:,8]=S1,
    sd[:,9]=S2, sd[:,10:14]=B1..4; sa[:,0]=A4, sa[:,1]=Am; sg[:,0]=Bm.
    """
    sds, sas, sgs = [], [], []
    for r in res.results:
        d = r["st_out"]
        m = _merge_slots(d[:, :, 0:5], np.maximum)
        rest = _merge_slots(d[:, :, 5:14], np.add)
        sds.append(np.concatenate([m, rest], 1))
        sas.append(_merge_slots(d[:, :, 14:16], np.add))
        sgs.append(_merge_slots(d[:, :, 16:17], np.add))
    return (
        np.concatenate(sds, 0),
        np.concatenate(sas, 0),
        np.concatenate(sgs, 0),
    )


def kernel(outputs1, outputs2, outputs3, outputs4, out_s, targets):
    global LAST_RESULTS
    outputs1 = np.asarray(outputs1, dtype=np.float32)
    outputs2 = np.asarray(outputs2, dtype=np.float32)
    outputs3 = np.asarray(outputs3, dtype=np.float32)
    outputs4 = np.asarray(outputs4, dtype=np.float32)
    out_s = np.asarray(out_s, dtype=np.float32)
    targets = np.asarray(targets)
    nc = _get_nc()

    in_maps = []
    for k in range(NCORES):
        slc = slice(k * ROWS, (k + 1) * ROWS)
        in_maps.append(
            {
                "o1": np.ascontiguousarray(outputs1[slc]),
                "o2": np.ascontiguousarray(outputs2[slc]),
                "o3": np.ascontiguousarray(outputs3[slc]),
                "o4": np.ascontiguousarray(outputs4[slc]),
                "s": np.ascontiguousarray(out_s[slc]),
            }
        )

    def _run():
        try:
            return run_bass_kernel_spmd(
                nc, in_maps, core_ids=list(range(NCORES))
            )
        except ModuleNotFoundError:
            # BASS_TRACE set but this environment lacks the axon NTFF hook
            os.environ["BASS_NEVER_TRACE"] = "1"
            return run_bass_kernel_spmd(
                nc, in_maps, core_ids=list(range(NCORES))
            )

    res = None
    for attempt in range(3):
        try:
            res = _run()
            break
        except ModuleNotFoundError:
            raise
        except Exception:
            # transient accelerator faults have been observed lasting more
            # than one attempt; back off and retry before giving up
            if attempt == 2:
                raise
            time.sleep(15 * (attempt + 1))
    LAST_RESULTS = res

    sd, sa, sg = gather_stats(res)

    return _finalize(
        sd, sa, sg, outputs1, outputs2, outputs3, outputs4, out_s, targets
    )


def _finalize(sd, sa, sg, outputs1, outputs2, outputs3, outputs4, out_s, targets):
    f32 = np.float32
    tgt = np.asarray(targets).astype(np.int64)
    ar = np.arange(B)

    M = sd[:, 0:5].astype(np.float64)  # maxes of bf16 exps (f32 exact values)
    A = np.stack(
        [sd[:, 5], sd[:, 6], sd[:, 7], sa[:, 0], sa[:, 1]], 1
    ).astype(np.float64)
    S1 = sd[:, 8].astype(np.float64)
    S2 = sd[:, 9].astype(np.float64)
    Bt = np.concatenate([sd[:, 10:14], sg[:, 0:1]], 1).astype(np.float64)

    # target-gathered logits (exact input f32 values)
    v1 = outputs1[ar, tgt]
    v2 = outputs2[ar, tgt]
    v3 = outputs3[ar, tgt]
    v4 = outputs4[ar, tgt]
    vs = out_s[ar, tgt]
    v5sum = ((v1 + v2) + v3) + v4  # f32 assoc matches device pm exactly
    v5 = v5sum * f32(0.25)

    teacher_arrs = (outputs1, outputs2, outputs3, outputs4)
    # device exp-domain values for the candidate tests
    ev = np.stack(
        [np.exp(v1.astype(np.float64) / T_KD),
         np.exp(v2.astype(np.float64) / T_KD),
         np.exp(v3.astype(np.float64) / T_KD),
         np.exp(v4.astype(np.float64) / T_KD),
         np.exp(v5sum.astype(np.float64) / (4.0 * T_KD))], 1
    )
    DELTA = 0.006  # covers bf16 rounding (2^-8) + exp ulp slack

    # margins: nonzero only where the target hits the row max. The device
    # maxes are bf16-domain, so over-approximate the hit set and recompute
    # those rows exactly on host (reference f32 semantics).
    margins = np.zeros((B, 5), np.float32)
    cand_r, cand_t = np.nonzero(ev * (1.0 + DELTA) >= M)
    for r, t in zip(cand_r, cand_t):
        if t < 4:
            row = teacher_arrs[t][r]
            tv = row[tgt[r]]
        else:
            row = (
                ((outputs1[r] + outputs2[r]) + outputs3[r]) + outputs4[r]
            ) * f32(0.25)
            tv = v5[r]
        top1 = row.max()
        if tv == top1:
            m2 = np.partition(row, -2)[-2]
            margins[r, t] = top1 - m2

    z = margins.astype(np.float64) / T_THR
    ez = np.exp(z - z.max(1, keepdims=True))
    thr = ez / ez.sum(1, keepdims=True)

    # exact global max_preds from bf16-domain bounds: candidate rows whose
    # upper bound reaches the best lower bound get an exact host max.
    logM = T_KD * np.log(M[:, 0:4])  # [B,4] approx row maxes, +-T_KD*DELTA
    glb = (logM - T_KD * DELTA).max()
    rr, tt = np.nonzero(logM + T_KD * DELTA >= glb)
    max_preds = np.float64(
        max(teacher_arrs[t][r].max() for r, t in zip(rr, tt))
    )

    vall = np.stack([v1, v2, v3, v4, v5], 1)  # [B,5] f32
    w = vall.astype(np.float64) / max_preds
    w1 = 1.0 - ALPHA * w
    w2 = ALPHA * w

    ce = np.log(S1) - vs.astype(np.float64)  # [B]
    kd = (T_KD * T_KD) * np.log(S2)[:, None] - T_KD * (Bt / A)  # [B,5]

    loss = w1 * ce[:, None] + w2 * kd
    per_sample = (thr * loss).sum(1)
    return np.asarray(per_sample.mean(), dtype=np.float32)


# revision 22
# speedup vs baseline: 1.1352x; 1.1352x over previous
"""Trainium2 Bass kernel for the Dynamic MultiTeacher4 distillation loss.

Strategy (pure data parallel over the batch):
  - B=8192 rows are sharded 1024/core across 8 NeuronCores; 8 row-blocks
    of 128 partitions per core.
  - Per row the device emits 17 reduction stats:
      M1..M4, Mm : row maxes of bf16(exp(o_t/20)) and bf16(exp(mimic4/80)).
                   exp is monotone, so the host recovers EXACT margin /
                   max_preds semantics from these with a small-tolerance
                   candidate search (a handful of rows recomputed exactly
                   on host).
      A1..A4, Am : sum_c exp(t/20)  (A4, Am ride free on ACT accum_out;
                   A1..A3 are DVE bf16 tensor_scalar sum-reduces at 4x)
      B1..B4, Bm : sum_c exp(t/20)*s  (bf16 DVE mult at 2x + bf16
                   tensor_scalar sum-reduce at 4x; the mimic dot usually
                   runs on GPSIMD, one entry lagged, to unload DVE)
      S1, S2     : sum_c exp(s), sum_c exp(s/20)
  - ACT cost is elems*0.83ns + ~370ns/instruction, so the teacher exps,
    exp(s/20) and exp(s) are fused into ONE wide activation over a
    [128, 5w] concatenated tile [o1|o2|o3|s|sk] at a single scale 1/20:
    GPSIMD precomputes sk = 20*s so exp(sk/20) == exp(s). e4 stays a
    separate instruction to keep its free accum (A4), as does
    em = exp(pm/80) (accum Am).
  - mimic4 = ((o1+o2)+o3)+o4 runs on the TensorEngine as 4 identity
    matmuls accumulating into PSUM, with operands bitcast to float32r:
    1 cycle/row instead of f32's 4 (the interpreter computes f32r
    exactly like f32, so pm stays bit-exact).
  - Block 0 uses per-tensor activations (baseline structure) so compute
    starts ~2.6us in instead of waiting for 4 tensors + a GPSIMD pass;
    the final block is processed in column halves to shorten the
    DMA-end -> stats critical chain.

Per-core HBM traffic is the 20.5MB of inputs; DMA is the roofline at
~57us and every compute engine is scheduled under it.
"""

import os
import time

import numpy as np

import concourse.bass as bass
import concourse.bacc as bacc
import concourse.tile as tile
from concourse import mybir
from concourse.bass_utils import run_bass_kernel_spmd
from concourse.masks import make_identity

B, C = 8192, 1000
NCORES = 8
ROWS = B // NCORES  # 1024 rows per core
P = 128
NBLK = ROWS // P  # 8 row-blocks per core

ALPHA = 0.8
T_KD = 20.0
T_THR = 2.0

# ramp/tail shaping (tunable)
SEP_N = int(os.environ.get("KERNEL_SEP_N", "1"))  # leading blocks in separate-exp mode
TAIL_HALVES = int(os.environ.get("KERNEL_TAIL_HALVES", "1"))  # trailing blocks split into C-halves

_NC = None
LAST_RESULTS = None  # BassKernelResults of the most recent run (for profiling)

# tensor order inside the concatenated tile: [o1|o2|o3|s|sk|o4]
SL_O1, SL_O2, SL_O3, SL_S, SL_SK, SL_O4 = range(6)
# st_dve columns
D_M = 0  # 0..4: M1..M4, Mm
D_A = 5  # 5..7: A1..A3
D_S1 = 8
D_S2 = 9
D_B = 10  # 10..13: B1..B4


def _entries():
    """(row_block, c0, width, slot, mode) for every virtual block."""
    ents = []
    slot = 0
    for i in range(NBLK):
        mode = "sep" if i < SEP_N else "wide"
        if i >= NBLK - TAIL_HALVES and mode == "wide":
            ents.append((i, 0, C // 2, slot, mode))
            ents.append((i, C // 2, C // 2, slot + 1, mode))
            slot += 2
        else:
            ents.append((i, 0, C, slot, mode))
            slot += 1
    return ents


ENTRIES = _entries()
NSLOT = len(ENTRIES)


def _build():
    f32 = mybir.dt.float32
    f32r = mybir.dt.float32r
    bf16 = mybir.dt.bfloat16
    Alu = mybir.AluOpType
    Act = mybir.ActivationFunctionType

    nc = bacc.Bacc(
        "TRN2", target_bir_lowering=False, debug=False, num_devices=NCORES
    )

    o1 = nc.dram_tensor("o1", [ROWS, C], f32, kind="ExternalInput").ap()
    o2 = nc.dram_tensor("o2", [ROWS, C], f32, kind="ExternalInput").ap()
    o3 = nc.dram_tensor("o3", [ROWS, C], f32, kind="ExternalInput").ap()
    o4 = nc.dram_tensor("o4", [ROWS, C], f32, kind="ExternalInput").ap()
    s_ = nc.dram_tensor("s", [ROWS, C], f32, kind="ExternalInput").ap()
    st_out = nc.dram_tensor("st_out", [NSLOT, P, 17], f32, kind="ExternalOutput").ap()

    o1r = o1.rearrange("(n p) c -> n p c", p=P)
    o2r = o2.rearrange("(n p) c -> n p c", p=P)
    o3r = o3.rearrange("(n p) c -> n p c", p=P)
    o4r = o4.rearrange("(n p) c -> n p c", p=P)
    sr = s_.rearrange("(n p) c -> n p c", p=P)
    dram = {SL_O1: o1r, SL_O2: o2r, SL_O3: o3r, SL_O4: o4r, SL_S: sr}

    with tile.TileContext(nc) as tc:
        with (
            tc.tile_pool(name="const", bufs=1) as const,
            tc.tile_pool(name="io", bufs=4) as io,
            tc.tile_pool(name="wk", bufs=3) as wk,
            tc.tile_pool(name="st", bufs=NSLOT + 1) as st,
            tc.tile_pool(name="ps", bufs=3, space="PSUM") as ps,
        ):
            ident = const.tile([P, P], f32, tag="ident")
            make_identity(nc, ident)
            # the BIR verifier requires fp32r matmul operands to come from
            # fp32r-typed producers
            identr = const.tile([P, P], f32r, tag="identr")
            nc.gpsimd.tensor_copy(out=identr, in_=ident)
            # warm the PE out of its cold p-state during the DMA-paced ramp
            warm = ps.tile(
                [P, 2, 500], f32, tag="pm2", padded_shape=[P, 2, 512], bufs=3
            )
            for _ in range(6):
                nc.tensor.matmul(
                    warm[:, 0, 0:P], ident, ident, start=True, stop=True
                )

            stats_tiles = []
            pending_pool_dots = []  # (in0, in1, accum_dst, width, tag)

            for i, c0, w, slot, mode in ENTRIES:
                nh = max(1, w // 500)
                H = w // nh
                # inA holds the 4 teachers (every writer fp32r-typed for the
                # BIR verifier's fp32r-matmul producer check; the DMA bitcasts
                # are pure bit copies so the f32 values are untouched).
                # inB = [s|sk] has no matmul consumers, so GPSIMD may write sk.
                inA = io.tile([P, 4 * w], f32, tag=f"inA{w}")
                inB = io.tile([P, 2 * w], f32, tag=f"inB{w}")
                ecA = wk.tile([P, 4 * w], bf16, tag=f"ecA{w}")
                ecB = wk.tile([P, 2 * w], bf16, tag=f"ecB{w}")
                stt = st.tile([P, 17], f32, tag="stt")
                sd = stt[:, 0:14]
                sa = stt[:, 14:16]
                sg = stt[:, 16:17]

                def slA(t, tile_=None):
                    tl = inA if tile_ is None else tile_
                    return tl[:, t * w : (t + 1) * w]

                s_in = inB[:, 0:w]
                sk_in = inB[:, w : 2 * w]
                es20 = ecB[:, 0:w]
                es = ecB[:, w : 2 * w]

                # ---- loads, issue spread over the SP/ACT/DVE queues so no
                # single sequencer's ~650ns/issue HWDGE occupancy paces the
                # stream ----
                if mode == "sep":
                    t_order = (0, None, 1, 2, 3)
                else:
                    t_order = (None, 0, 1, 2, 3)
                qs = (nc.sync, nc.sync, nc.sync, nc.sync, nc.sync)
                for q, t in zip(qs, t_order):
                    if t is None:
                        q.dma_start(out=s_in, in_=dram[SL_S][i][:, c0 : c0 + w])
                    else:
                        q.dma_start(
                            out=slA(t).bitcast(f32r),
                            in_=dram[t][i][:, c0 : c0 + w].bitcast(f32r),
                        )

                # ---- GPSIMD: sk = 20*s (wide mode), sb = bf16(s), and the
                # previous entry's lagged dot PRODUCTS (Pool has no accum
                # support in codegen; the paired reduces run on DVE below) ----
                sb = wk.tile([P, w], bf16, tag=f"sb{w}")
                if mode == "wide":
                    nc.gpsimd.tensor_scalar(
                        out=sk_in, in0=s_in, scalar1=20.0, scalar2=None,
                        op0=Alu.mult, op1=Alu.bypass,
                    )
                nc.gpsimd.tensor_copy(out=sb, in_=s_in)
                lagged_reduces = []  # (product_tile, accum_slice)
                for em_p, sb_p, dst_p, w_p, tagn in pending_pool_dots:
                    pp = wk.tile([P, w_p], bf16, tag=f"pp{tagn}{w_p}", bufs=2)
                    nc.gpsimd.tensor_mul(out=pp, in0=em_p, in1=sb_p)
                    lagged_reduces.append((pp, dst_p))
                pending_pool_dots = []

                # ---- PE: pm = ((o1+o2)+o3)+o4 in f32r (exact in interp) ----
                pm = ps.tile(
                    [P, nh, H], f32, tag=f"pm{nh}",
                    padded_shape=[P, nh, 512], bufs=(3 if nh == 2 else 2),
                )
                for j in range(nh):
                    for k in range(4):
                        nc.tensor.matmul(
                            pm[:, j, :],
                            identr,
                            inA[:, k * w + j * H : k * w + (j + 1) * H].bitcast(f32r),
                            start=(k == 0),
                            stop=(k == 3),
                        )

                # ---- ACT ----
                em = wk.tile([P, w], bf16, tag=f"em{w}")
                if mode == "sep":
                    # per-tensor exps, accum sums ride free
                    for k in range(3):
                        nc.scalar.activation(
                            out=slA(k, ecA), in_=slA(k), func=Act.Exp,
                            scale=1.0 / T_KD,
                            accum_out=sd[:, D_A + k : D_A + k + 1],
                        )
                    nc.scalar.activation(
                        out=slA(3, ecA), in_=slA(3), func=Act.Exp,
                        scale=1.0 / T_KD, accum_out=sa[:, 0:1],
                    )
                    nc.scalar.activation(
                        out=em.rearrange("p (j c) -> p j c", j=nh), in_=pm,
                        func=Act.Exp, scale=1.0 / (4.0 * T_KD),
                        accum_out=sa[:, 1:2],
                    )
                    nc.scalar.activation(
                        out=es20, in_=s_in, func=Act.Exp,
                        scale=1.0 / T_KD, accum_out=sd[:, D_S2 : D_S2 + 1],
                    )
                    nc.scalar.activation(
                        out=es, in_=s_in, func=Act.Exp,
                        scale=1.0, accum_out=sd[:, D_S1 : D_S1 + 1],
                    )
                else:
                    # wide exp over [o1|o2|o3] and [s|sk], both at scale 1/20
                    nc.scalar.activation(
                        out=ecA[:, 0 : 3 * w], in_=inA[:, 0 : 3 * w],
                        func=Act.Exp, scale=1.0 / T_KD,
                    )
                    nc.scalar.activation(
                        out=ecB, in_=inB, func=Act.Exp, scale=1.0 / T_KD,
                    )
                    nc.scalar.activation(
                        out=slA(3, ecA), in_=slA(3), func=Act.Exp,
                        scale=1.0 / T_KD, accum_out=sa[:, 0:1],
                    )
                    nc.scalar.activation(
                        out=em.rearrange("p (j c) -> p j c", j=nh), in_=pm,
                        func=Act.Exp, scale=1.0 / (4.0 * T_KD),
                        accum_out=sa[:, 1:2],
                    )

                # ---- DVE ----
                scrap = wk.tile([P, w], bf16, tag=f"sc{w}", bufs=2)
                prod = wk.tile([P, w], bf16, tag=f"pr{w}", bufs=2)

                def dve_max(src, col):
                    nc.vector.tensor_scalar(
                        out=scrap, in0=src, scalar1=1.0, scalar2=None,
                        op0=Alu.mult, op1=Alu.max,
                        accum_out=sd[:, col : col + 1],
                    )

                def dve_sum(src, dst):
                    nc.vector.tensor_scalar(
                        out=scrap, in0=src, scalar1=1.0, scalar2=None,
                        op0=Alu.mult, op1=Alu.add, accum_out=dst,
                    )

                def dve_dot(src, col):
                    nc.vector.tensor_mul(out=prod, in0=src, in1=sb)
                    nc.vector.tensor_scalar(
                        out=scrap, in0=prod, scalar1=1.0, scalar2=None,
                        op0=Alu.mult, op1=Alu.add,
                        accum_out=sd[:, col : col + 1],
                    )

                # previous entry's lagged reduces first (their Pool products
                # are ready early in this block; widths may differ from w)
                for pp, dst in lagged_reduces:
                    w_p = pp.shape[1]
                    scl = wk.tile([P, w_p], bf16, tag=f"scl{w_p}", bufs=2)
                    nc.vector.tensor_scalar(
                        out=scl, in0=pp, scalar1=1.0, scalar2=None,
                        op0=Alu.mult, op1=Alu.add, accum_out=dst,
                    )

                for k in range(3):
                    dve_max(slA(k, ecA), D_M + k)
                    if mode == "wide":
                        dve_sum(slA(k, ecA), sd[:, D_A + k : D_A + k + 1])
                    dve_dot(slA(k, ecA), D_B + k)
                if mode == "wide":
                    dve_sum(es, sd[:, D_S1 : D_S1 + 1])
                    dve_sum(es20, sd[:, D_S2 : D_S2 + 1])
                dve_max(slA(3, ecA), D_M + 3)
                last = slot == NSLOT - 1
                if not last and slot % 2 == 0:
                    # lag the e4 dot product onto Pool on even slots
                    pending_pool_dots.append(
                        (slA(3, ecA), sb, sd[:, D_B + 3 : D_B + 4], w, "e4")
                    )
                else:
                    dve_dot(slA(3, ecA), D_B + 3)
                dve_max(em, D_M + 4)
                if last:
                    # tail: run the mimic dot on DVE instead of lagging it
                    nc.vector.tensor_mul(out=prod, in0=em, in1=sb)
                    nc.vector.tensor_scalar(
                        out=scrap, in0=prod, scalar1=1.0, scalar2=None,
                        op0=Alu.mult, op1=Alu.add, accum_out=sg[:, 0:1],
                    )
                else:
                    pending_pool_dots.append((em, sb, sg[:, 0:1], w, "m"))

                stats_tiles.append((slot, stt))

            # stats stores after the loop: never block a later block's loads
            # behind a store on the in-order sync queue
            for slot, stt in stats_tiles:
                nc.sync.dma_start(out=st_out[slot], in_=stt)

    nc.compile()
    return nc


def _get_nc():
    global _NC
    if _NC is None:
        _NC = _build()
    return _NC


def _merge_slots(arr, op):
    """[NSLOT, P, K] per-slot stats -> [NBLK*P, K] per-row stats."""
    out = []
    for i in range(NBLK):
        slots = [s for (ib, _c0, _w, s, _m) in ENTRIES if ib == i]
        m = arr[slots[0]]
        for s in slots[1:]:
            m = op(m, arr[s])
        out.append(m)
    return np.concatenate(out, 0)


def gather_stats(res):
    """Merge per-slot device stats into per-row [B, *] arrays.

    Returns (sd, sa, sg): sd[:,0:5]=M maxes, sd[:,5:8]=A1..3, sd[:,8]=S1,
    sd[:,9]=S2, sd[:,10:14]=B1..4; sa[:,0]=A4, sa[:,1]=Am; sg[:,0]=Bm.
    """
    sds, sas, sgs = [], [], []
    for r in res.results:
        d = r["st_out"]
        m = _merge_slots(d[:, :, 0:5], np.maximum)
        rest = _merge_slots(d[:, :, 5:14], np.add)
        sds.append(np.concatenate([m, rest], 1))
        sas.append(_merge_slots(d[:, :, 14:16], np.add))
        sgs.append(_merge_slots(d[:, :, 16:17], np.add))
    return (
        np.concatenate(sds, 0),
        np.concatenate(sas, 0),
        np.concatenate(sgs, 0),
    )


def kernel(outputs1, outputs2, outputs3, outputs4, out_s, targets):
    global LAST_RESULTS
    outputs1 = np.asarray(outputs1, dtype=np.float32)
    outputs2 = np.asarray(outputs2, dtype=np.float32)
    outputs3 = np.asarray(outputs3, dtype=np.float32)
    outputs4 = np.asarray(outputs4, dtype=np.float32)
    out_s = np.asarray(out_s, dtype=np.float32)
    targets = np.asarray(targets)
    nc = _get_nc()

    in_maps = []
    for k in range(NCORES):
        slc = slice(k * ROWS, (k + 1) * ROWS)
        in_maps.append(
            {
                "o1": np.ascontiguousarray(outputs1[slc]),
                "o2": np.ascontiguousarray(outputs2[slc]),
                "o3": np.ascontiguousarray(outputs3[slc]),
                "o4": np.ascontiguousarray(outputs4[slc]),
                "s": np.ascontiguousarray(out_s[slc]),
            }
        )

    def _run():
        try:
            return run_bass_kernel_spmd(
                nc, in_maps, core_ids=list(range(NCORES))
            )
        except ModuleNotFoundError:
            # BASS_TRACE set but this environment lacks the axon NTFF hook
            os.environ["BASS_NEVER_TRACE"] = "1"
            return run_bass_kernel_spmd(
                nc, in_maps, core_ids=list(range(NCORES))
            )

    res = None
    for attempt in range(3):
        try:
            res = _run()
            break
        except ModuleNotFoundError:
            raise
        except Exception:
            # transient accelerator faults have been observed lasting more
            # than one attempt; back off and retry before giving up
            if attempt == 2:
                raise
            time.sleep(15 * (attempt + 1))
    LAST_RESULTS = res

    sd, sa, sg = gather_stats(res)

    return _finalize(
        sd, sa, sg, outputs1, outputs2, outputs3, outputs4, out_s, targets
    )


def _finalize(sd, sa, sg, outputs1, outputs2, outputs3, outputs4, out_s, targets):
    f32 = np.float32
    tgt = np.asarray(targets).astype(np.int64)
    ar = np.arange(B)

    M = sd[:, 0:5].astype(np.float64)  # maxes of bf16 exps (f32 exact values)
    A = np.stack(
        [sd[:, 5], sd[:, 6], sd[:, 7], sa[:, 0], sa[:, 1]], 1
    ).astype(np.float64)
    S1 = sd[:, 8].astype(np.float64)
    S2 = sd[:, 9].astype(np.float64)
    Bt = np.concatenate([sd[:, 10:14], sg[:, 0:1]], 1).astype(np.float64)

    # target-gathered logits (exact input f32 values)
    v1 = outputs1[ar, tgt]
    v2 = outputs2[ar, tgt]
    v3 = outputs3[ar, tgt]
    v4 = outputs4[ar, tgt]
    vs = out_s[ar, tgt]
    v5sum = ((v1 + v2) + v3) + v4  # f32 assoc matches device pm exactly
    v5 = v5sum * f32(0.25)

    teacher_arrs = (outputs1, outputs2, outputs3, outputs4)
    # device exp-domain values for the candidate tests
    ev = np.stack(
        [np.exp(v1.astype(np.float64) / T_KD),
         np.exp(v2.astype(np.float64) / T_KD),
         np.exp(v3.astype(np.float64) / T_KD),
         np.exp(v4.astype(np.float64) / T_KD),
         np.exp(v5sum.astype(np.float64) / (4.0 * T_KD))], 1
    )
    DELTA = 0.006  # covers bf16 rounding (2^-8) + exp ulp slack

    # margins: nonzero only where the target hits the row max. The device
    # maxes are bf16-domain, so over-approximate the hit set and recompute
    # those rows exactly on host (reference f32 semantics).
    margins = np.zeros((B, 5), np.float32)
    cand_r, cand_t = np.nonzero(ev * (1.0 + DELTA) >= M)
    for r, t in zip(cand_r, cand_t):
        if t < 4:
            row = teacher_arrs[t][r]
            tv = row[tgt[r]]
        else:
            row = (
                ((outputs1[r] + outputs2[r]) + outputs3[r]) + outputs4[r]
            ) * f32(0.25)
            tv = v5[r]
        top1 = row.max()
        if tv == top1:
            m2 = np.partition(row, -2)[-2]
            margins[r, t] = top1 - m2

    z = margins.astype(np.float64) / T_THR
    ez = np.exp(z - z.max(1, keepdims=True))
    thr = ez / ez.sum(1, keepdims=True)

    # exact global max_preds from bf16-domain bounds: candidate rows whose
    # upper bound reaches the best lower bound get an exact host max.
    logM = T_KD * np.log(M[:, 0:4])  # [B,4] approx row maxes, +-T_KD*DELTA
    glb = (logM - T_KD * DELTA).max()
    rr, tt = np.nonzero(logM + T_KD * DELTA >= glb)
    max_preds = np.float64(
        max(teacher_arrs[t][r].max() for r, t in zip(rr, tt))
    )

    vall = np.stack([v1, v2, v3, v4, v5], 1)  # [B,5] f32
    w = vall.astype(np.float64) / max_preds
    w1 = 1.0 - ALPHA * w
    w2 = ALPHA * w

    ce = np.log(S1) - vs.astype(np.float64)  # [B]
    kd = (T_KD * T_KD) * np.log(S2)[:, None] - T_KD * (Bt / A)  # [B,5]

    loss = w1 * ce[:, None] + w2 * kd
    per_sample = (thr * loss).sum(1)
    return np.asarray(per_sample.mean(), dtype=np.float32)
